# revision 38
# baseline (speedup 1.0000x reference)
"""Self-contained Trainium2 kernel for nn_AutoregressiveGroupQuerySelfAttention.

Reference computation (B=2, S=2048, H=2048, 16 heads x 128 dim):
    q = (x @ Wq.T) -> heads; k likewise; v likewise
    q, k get RoPE; scores = (q @ k.T) * sqrt(D)   (faithful-to-source bug)
    causal softmax; ctx = attn @ v; out = ctx @ Wo.T

Sharding over 8 NeuronCores: core c = (b, g) with b = c // 4 (batch),
g = c % 4 (head-group of 4 heads = 512 hidden columns).  Each core computes
its head-group's context and a partial output  ctx_g @ Wo.T[g-rows, :];
the host sums the 4 partials per batch element.

Attention avoids PE transposes entirely: scores are computed directly in
transposed orientation S^T[k, q] = krope_t^T @ qrope (contraction over the
head dim on partitions), so the P^T operand for the PV matmul comes straight
out of the exp.  The softmax max is estimated from a contiguous half of the
key range (exact for the causally-short rows); a -32 margin makes overflow
of exp(s - m_hat - 32) astronomically unlikely while the common scale
cancels in P/l.  The -(m_hat+32) bias is applied by a rank-1 matmul fused
into each score accumulation group; row sums l accumulate on the vector
engine and normalization uses a ones-broadcast matmul.
"""
import numpy as np
import ml_dtypes

import concourse.bass as bass
import concourse.mybir as mybir
from concourse import bacc
from concourse.tile import TileContext
from concourse.bass_utils import run_bass_kernel_spmd

F32 = mybir.dt.float32
F32R = mybir.dt.float32r
BF16 = mybir.dt.bfloat16
AX = mybir.AxisListType
ALU = mybir.AluOpType
ACTF = mybir.ActivationFunctionType

B, S, H = 2, 2048, 2048
NUM_HEADS, D = 16, 128
N_CORES = 8
NH = 4                     # heads per core
HG = NH * D                # 512
ROPE_BASE = 10000.0
MARGIN = 32.0              # exp bias margin; cancels in P/l
FULL_MAX = True           # exact max over all causal keys (debug/safety)

_NC_CACHE = {}
LAST_RESULTS = None        # BassKernelResults of the most recent run (for profiling)
TRACE = False


def _build(S_=S, H_=H, NH_=NH):
    DD = 128
    HG_ = NH_ * DD
    KT = H_ // 128
    SQT = S_ // 128
    CH = 512
    NCHUNK = S_ // CH

    nc = bacc.Bacc()
    xT = nc.declare_dram_parameter("xT", [H_, S_], F32R, isOutput=False)
    xbfT = nc.declare_dram_parameter("xbfT", [H_, S_], BF16, isOutput=False)
    wqT = nc.declare_dram_parameter("wqT", [H_, HG_], F32R, isOutput=False)
    wkT = nc.declare_dram_parameter("wkT", [H_, HG_], F32R, isOutput=False)
    wvT = nc.declare_dram_parameter("wvT", [H_, HG_], BF16, isOutput=False)
    woT = nc.declare_dram_parameter("woT", [HG_, H_], BF16, isOutput=False)
    cosT = nc.declare_dram_parameter("cosT", [128, S_], F32, isOutput=False)
    sinT = nc.declare_dram_parameter("sinT", [128, S_], F32, isOutput=False)
    rT = nc.declare_dram_parameter("rT", [128, 128], F32R, isOutput=False)
    onesf = nc.declare_dram_parameter("onesf", [1, 128], BF16, isOutput=False)
    negmarg = nc.declare_dram_parameter("negmarg", [128, 1], F32, isOutput=False)
    onescol = nc.declare_dram_parameter("onescol", [128, 1], F32R, isOutput=False)
    oneskk = nc.declare_dram_parameter("oneskk", [128, 128], BF16, isOutput=False)
    mask = nc.declare_dram_parameter("mask", [128, 128], F32, isOutput=False)
    maskT = nc.declare_dram_parameter("maskT", [128, 128], F32, isOutput=False)
    out = nc.declare_dram_parameter("out", [S_, H_], F32, isOutput=True)

    with TileContext(nc) as tc:
        with (
            tc.tile_pool(name="slabs", bufs=1) as slabp,
            tc.tile_pool(name="stats", bufs=8) as statp,
            tc.tile_pool(name="consts", bufs=1) as constp,
            tc.tile_pool(name="psbig", bufs=3, space="PSUM") as psbig,
            tc.tile_pool(name="pssmall", bufs=3, space="PSUM") as pssmall,
            tc.tile_pool(name="psctx", bufs=2, space="PSUM") as psctx,
        ):
            onesf_sb = constp.tile([1, 128], BF16, tag="onesf")
            nc.sync.dma_start(out=onesf_sb[:], in_=onesf[:])
            negmarg_sb = constp.tile([128, 1], F32, tag="negmarg")
            nc.sync.dma_start(out=negmarg_sb[:], in_=negmarg[:])
            onescol_sb = constp.tile([128, 1], F32R, tag="onescol")
            nc.sync.dma_start(out=onescol_sb[:], in_=onescol[:])
            oneskk_sb = constp.tile([128, 128], BF16, tag="oneskk")
            nc.sync.dma_start(out=oneskk_sb[:], in_=oneskk[:])
            mask_sb = constp.tile([128, 128], F32, tag="mask")
            nc.sync.dma_start(out=mask_sb[:], in_=mask[:])
            maskT_sb = constp.tile([128, 128], F32, tag="maskT")
            nc.sync.dma_start(out=maskT_sb[:], in_=maskT[:])
            qrope = [slabp.tile([128, S_], F32R, tag=f"qrope{h}", name=f"qrope{h}") for h in range(NH_)]
            krope = [slabp.tile([128, S_], F32R, tag=f"krope{h}", name=f"krope{h}") for h in range(NH_)]
            vslab = slabp.tile([128, SQT * HG_], BF16, tag="vslab")

            # ====== era 1: q/k projections + RoPE ======
            with (
                tc.tile_pool(name="w1", bufs=1) as wp1,
                tc.tile_pool(name="xin1", bufs=1) as xp1,
                tc.tile_pool(name="tab", bufs=2) as tabp,
                tc.tile_pool(name="work", bufs=2) as workp,
            ):
                rT_sb = wp1.tile([128, 128], F32R, tag="rT")
                nc.sync.dma_start(out=rT_sb[:], in_=rT[:])
                wq_sb = wp1.tile([128, KT * HG_], F32R, tag="wq")
                nc.sync.dma_start(
                    out=wq_sb[:].rearrange("p (kt j) -> p kt j", kt=KT),
                    in_=wqT.rearrange("(kt p) j -> p kt j", p=128),
                )
                wk_sb = wp1.tile([128, KT * HG_], F32R, tag="wk")
                nc.sync.dma_start(
                    out=wk_sb[:].rearrange("p (kt j) -> p kt j", kt=KT),
                    in_=wkT.rearrange("(kt p) j -> p kt j", p=128),
                )

                xT3 = xT.rearrange("(kt p) s -> p kt s", p=128)
                for sc in range(NCHUNK):
                    cs = slice(sc * CH, (sc + 1) * CH)
                    cos_t = tabp.tile([128, CH], F32, tag="cos")
                    nc.sync.dma_start(out=cos_t[:], in_=cosT[:, cs])
                    sin_t = tabp.tile([128, CH], F32, tag="sin")
                    nc.sync.dma_start(out=sin_t[:], in_=sinT[:, cs])
                    xk = []
                    for kt in range(KT):
                        t = xp1.tile([128, CH], F32R, tag=f"xb{kt}", name=f"xb{kt}")
                        nc.sync.dma_start(out=t[:], in_=xT3[:, kt, cs])
                        xk.append(t)
                    pending = None

                    def finish_rope(raw, ropes, h):
                        rotps = pssmall.tile([128, CH], F32, tag="small", name="rotps")
                        nc.tensor.matmul(rotps[:], rT_sb[:], raw[:], start=True, stop=True)
                        t1 = workp.tile([128, CH], F32, tag="t1", name="t1")
                        nc.vector.tensor_mul(t1[:], rotps[:], sin_t[:])
                        t2 = workp.tile([128, CH], F32, tag="t2", name="t2")
                        nc.vector.tensor_mul(t2[:], raw[:].bitcast(F32), cos_t[:])
                        nc.vector.tensor_add(ropes[h][:, cs], t1[:], t2[:])

                    for w_sb, ropes in ((wq_sb, qrope), (wk_sb, krope)):
                        for h in range(NH_):
                            ps = psbig.tile([128, CH], F32, tag="big")
                            for kt in range(KT):
                                nc.tensor.matmul(
                                    ps[:],
                                    w_sb[:, kt * HG_ + h * 128: kt * HG_ + (h + 1) * 128],
                                    xk[kt][:],
                                    start=(kt == 0),
                                    stop=(kt == KT - 1),
                                )
                            raw = workp.tile([128, CH], F32R, tag="raw")
                            nc.vector.tensor_copy(raw[:], ps[:])
                            if pending is not None:
                                finish_rope(*pending)
                            pending = (raw, ropes, h)
                    finish_rope(*pending)

            # ====== era 2: v projection, attention, output projection ======
            # Software-pipelined: sweep-1 (stats) runs 2 iterations ahead of
            # sweep-2 (S^T+exp+PV); normalization lags 1 iteration; the V
            # projection fills the pipe while the first stats chains drain.
            with (
                tc.tile_pool(name="w2", bufs=1) as wp2,
                tc.tile_pool(name="xin2", bufs=1) as xp2,
                tc.tile_pool(name="ptpool", bufs=5) as ptp,
                tc.tile_pool(name="lpool", bufs=2) as lp,
                tc.tile_pool(name="rowpool", bufs=8) as rowp,
                tc.tile_pool(name="ctxpool", bufs=1) as ctxp,
                tc.tile_pool(name="bcpool", bufs=2) as bcp,
                tc.tile_pool(name="bbpool", bufs=3) as bbp,
                tc.tile_pool(name="ostage", bufs=2) as ostp,
            ):
                wv_sb = wp2.tile([128, KT * HG_], BF16, tag="wv")
                nc.sync.dma_start(
                    out=wv_sb[:].rearrange("p (kt j) -> p kt j", kt=KT),
                    in_=wvT.rearrange("(kt p) j -> p kt j", p=128),
                )
                wo_sb = wp2.tile([128, NH_ * H_], BF16, tag="wo")
                nc.sync.dma_start(
                    out=wo_sb[:].rearrange("p (j ho) -> p j ho", j=NH_),
                    in_=woT.rearrange("(j p) ho -> p j ho", p=128),
                )

                ctxT = [ctxp.tile([128, S_], BF16, tag=f"ctxT{h}", name=f"ctxT{h}") for h in range(NH_)]
                NIT = NH_ * NCHUNK
                st = [dict() for _ in range(NIT)]

                def sweep1(it):
                    h, c = divmod(it, NCHUNK)
                    negm4 = statp.tile([128, 4], F32, tag="negm4", name=f"negm4_{it}")
                    for j in range(4):
                        sq = 4 * c + j
                        hs = min(4 * c + 4 if FULL_MAX else 2 * c + 2, sq + 1)
                        cols = hs * 128
                        nmm = (cols + CH - 1) // CH
                        mx2 = statp.tile([128, 4], F32, tag="mx2", name=f"mx2_{it}_{j}")
                        for m in range(nmm):
                            c0 = m * CH
                            c1 = min(cols, c0 + CH)
                            scps = psbig.tile([128, CH], F32, tag="big", name="scps")
                            nc.tensor.matmul(
                                scps[:, :c1 - c0],
                                qrope[h][:, sq * 128:(sq + 1) * 128],
                                krope[h][:, c0:c1],
                                start=True,
                                stop=True,
                            )
                            if sq < hs and m == sq // 4:
                                dcol = (sq % 4) * 128
                                nc.vector.tensor_add(
                                    scps[:, dcol:dcol + 128],
                                    scps[:, dcol:dcol + 128],
                                    mask_sb[:],
                                )
                            if nmm > 1:
                                nc.vector.tensor_reduce(
                                    mx2[:, m:m + 1], scps[:, :c1 - c0], axis=AX.X, op=ALU.max
                                )
                            else:
                                nc.vector.tensor_reduce(
                                    negm4[:, j:j + 1], scps[:, :c1 - c0],
                                    axis=AX.X, op=ALU.max, negate=True,
                                )
                        if nmm > 1:
                            nc.vector.tensor_reduce(
                                negm4[:, j:j + 1], mx2[:, :nmm], axis=AX.X, op=ALU.max, negate=True
                            )
                    negmf4 = statp.tile([128, 4], BF16, tag="negmf4", name=f"negmf4_{it}")
                    nc.vector.tensor_copy(negmf4[:], negm4[:])
                    brow = rowp.tile([1, CH], BF16, tag="brow", name=f"brow_{it}")
                    for j in range(4):
                        nc.sync.dma_start(
                            out=brow[0:1, j * 128:(j + 1) * 128], in_=negmf4[:, j:j + 1]
                        )
                    st[it]["brow"] = brow

                def sweep1_bb(it):
                    brow = st[it]["brow"]
                    bbps = psbig.tile([128, CH], F32, tag="big", name=f"bbps_{it}")
                    nc.tensor.matmul(bbps[:], onesf_sb[:], brow[:], start=True, stop=True)
                    bbsb = bbp.tile([128, CH], BF16, tag="bb", name=f"bb_{it}")
                    nc.scalar.copy(bbsb[:], bbps[:])
                    st[it]["bb"] = bbsb

                def sweep2(it):
                    h, c = divmod(it, NCHUNK)
                    bbsb = st[it]["bb"]
                    T = 4 * c + 4
                    ctxps = psctx.tile([128, CH], F32, tag="ctx", name=f"ctx_{it}")
                    Lsum = lp.tile([128, CH], F32R, tag="L", name=f"Lsum_{it}")
                    pend = []

                    def drain(item):
                        t, off, pt = item
                        nc.tensor.matmul(
                            ctxps[:, off:CH],
                            vslab[:, t * HG_ + h * 128: t * HG_ + (h + 1) * 128],
                            pt[:, off:CH],
                            start=(t == 0),
                            stop=(t == T - 1),
                        )
                        if t == 0:
                            nc.gpsimd.tensor_copy(Lsum[:], pt[:])
                        elif t % 3 == 2:
                            nc.vector.tensor_add(
                                Lsum[:, off:CH], Lsum[:, off:CH], pt[:, off:CH]
                            )
                        else:
                            nc.gpsimd.tensor_add(
                                Lsum[:, off:CH], Lsum[:, off:CH], pt[:, off:CH]
                            )

                    for t in range(T):
                        off = max(0, (t - 4 * c) * 128)
                        stps = pssmall.tile([128, CH], F32, tag="small", name="stps")
                        nc.tensor.matmul(
                            stps[:, off:CH],
                            krope[h][:, t * 128:(t + 1) * 128],
                            qrope[h][:, c * CH + off:(c + 1) * CH],
                            start=True,
                            stop=False,
                        )
                        nc.tensor.matmul(
                            stps[:, off:CH],
                            oneskk_sb[:],
                            bbsb[:, off:CH],
                            start=False,
                            stop=True,
                        )
                        if t >= 4 * c:
                            nc.vector.tensor_add(
                                stps[:, off:off + 128],
                                stps[:, off:off + 128],
                                maskT_sb[:],
                            )
                        pt = ptp.tile([128, CH], BF16, tag="pt", name=f"pt_{it}_{t}")
                        nc.scalar.activation(
                            pt[:, off:CH], stps[:, off:CH], ACTF.Exp, bias=negmarg_sb[:]
                        )
                        pend.append((t, off, pt))
                        if len(pend) > 3:
                            drain(pend.pop(0))
                    for item in pend:
                        drain(item)
                    st[it]["ctxps"] = ctxps
                    st[it]["Lsum"] = Lsum

                def lstage(it):
                    lrow = pssmall.tile([1, CH], F32, tag="small", name=f"lrow_{it}")
                    nc.tensor.matmul(lrow[:], onescol_sb[:], st[it]["Lsum"][:], start=True, stop=True)
                    rcprow = bcp.tile([1, CH], F32, tag="rcprow", name=f"rcprow_{it}")
                    nc.vector.reciprocal(rcprow[:], lrow[:])
                    rcpb = bcp.tile([1, CH], BF16, tag="rcpb", name=f"rcpb_{it}")
                    nc.vector.tensor_copy(rcpb[:], rcprow[:])
                    st[it]["rcpb"] = rcpb

                def norm(it):
                    h, c = divmod(it, NCHUNK)
                    bcps = pssmall.tile([128, CH], F32, tag="small", name=f"bcps_{it}")
                    nc.tensor.matmul(bcps[:], onesf_sb[:], st[it]["rcpb"][:], start=True, stop=True)
                    bcsb = bcp.tile([128, CH], F32, tag="bcsb", name=f"bcsb_{it}")
                    nc.vector.tensor_copy(bcsb[:], bcps[:])
                    nc.vector.tensor_mul(
                        ctxT[h][:, c * CH:(c + 1) * CH], st[it]["ctxps"][:], bcsb[:]
                    )
                    st[it].clear()
                    if h == NH_ - 1:
                        for sti in range(4 * c, 4 * c + 4):
                            ostg = ostp.tile([128, H_], F32, tag="ostg", name="ostg")
                            for hoc in range(H_ // CH):
                                wops = psbig.tile([128, CH], F32, tag="big", name="wops")
                                for j in range(NH_):
                                    nc.tensor.matmul(
                                        wops[:],
                                        ctxT[j][:, sti * 128:(sti + 1) * 128],
                                        wo_sb[:, j * H_ + hoc * CH: j * H_ + (hoc + 1) * CH],
                                        start=(j == 0),
                                        stop=(j == NH_ - 1),
                                    )
                                if hoc % 2 == 0:
                                    nc.scalar.copy(ostg[:, hoc * CH:(hoc + 1) * CH], wops[:])
                                else:
                                    nc.vector.tensor_copy(ostg[:, hoc * CH:(hoc + 1) * CH], wops[:])
                            nc.sync.dma_start(out=out[sti * 128:(sti + 1) * 128, :], in_=ostg[:])

                xbf3 = xbfT.rearrange("(kt p) s -> p kt s", p=128)

                vtiles = {}

                def vdma(sc):
                    cs = slice(sc * CH, (sc + 1) * CH)
                    xkv = []
                    for kt in range(KT):
                        tile = xp2.tile([128, CH], BF16, tag=f"xv{kt}", name=f"xv{kt}_{sc}")
                        nc.sync.dma_start(out=tile[:], in_=xbf3[:, kt, cs])
                        xkv.append(tile)
                    vtiles[sc] = xkv

                def vchunk(sc):
                    xkv = vtiles.pop(sc)
                    for tl in range(4):
                        t = 4 * sc + tl
                        vps = psbig.tile([128, HG_], F32, tag="big")
                        for kt in range(KT):
                            nc.tensor.matmul(
                                vps[:],
                                xkv[kt][:, tl * 128:(tl + 1) * 128],
                                wv_sb[:, kt * HG_:(kt + 1) * HG_],
                                start=(kt == 0),
                                stop=(kt == KT - 1),
                            )
                        if t % 2 == 0:
                            nc.scalar.copy(vslab[:, t * HG_:(t + 1) * HG_], vps[:])
                        else:
                            nc.vector.tensor_copy(vslab[:, t * HG_:(t + 1) * HG_], vps[:])

                # pipeline fill: stats for the first six iterations + V chunk 0
                for it in range(6):
                    sweep1(it)
                sweep1_bb(0)
                sweep1_bb(1)
                sweep1_bb(2)
                vdma(0)
                vdma(1)
                vchunk(0)
                sweep1_bb(3)
                sweep1_bb(4)
                for it in range(NIT):
                    sweep2(it)
                    if it + 2 < NCHUNK:
                        vdma(it + 2)
                    if it + 1 < NCHUNK:
                        vchunk(it + 1)
                    if it + 6 < NIT:
                        sweep1(it + 6)
                    if it + 5 < NIT:
                        sweep1_bb(it + 5)
                    lstage(it)
                    if it >= 1:
                        norm(it - 1)
                norm(NIT - 1)

    nc.compile()
    return nc


def _make_tables(S_, D_=128):
    inv_freq = 1.0 / (ROPE_BASE ** (np.arange(0, D_, 2, dtype=np.float32) / D_))
    pos = np.arange(S_, dtype=np.float32)
    ang = pos[:, None] * inv_freq[None, :]
    ang = np.concatenate([ang, ang], axis=1)
    return (
        np.cos(ang).T.astype(np.float32).copy(),
        np.sin(ang).T.astype(np.float32).copy(),
    )


def _make_rot_T(D_=128):
    R = np.zeros((D_, D_), dtype=np.float32)
    half = D_ // 2
    for d in range(half):
        R[d, d + half] = -1.0
    for d in range(half, D_):
        R[d, d - half] = 1.0
    return R.T.copy()


def _make_mask(mask_val=-1e30):
    m = np.zeros((128, 128), dtype=np.float32)
    m[np.triu_indices(128, k=1)] = mask_val
    return m


def kernel(x, Wq, Wk, Wv, Wo):
    """Full inputs in, full output out. Shards over 8 NeuronCores internally."""
    global LAST_RESULTS
    x = np.ascontiguousarray(np.asarray(x, dtype=np.float32))
    Wq = np.asarray(Wq, dtype=np.float32)
    Wk = np.asarray(Wk, dtype=np.float32)
    Wv = np.asarray(Wv, dtype=np.float32)
    Wo = np.asarray(Wo, dtype=np.float32)

    if "nc" not in _NC_CACHE:
        _NC_CACHE["nc"] = _build()
    nc = _NC_CACHE["nc"]

    scale = np.sqrt(np.float32(D))
    cosT, sinT = _make_tables(S)
    rT = _make_rot_T()
    onesf = np.ones((1, 128), dtype=ml_dtypes.bfloat16)
    onescol = np.ones((128, 1), dtype=np.float32)
    oneskk = np.full((128, 128), 1.0 / 128.0, dtype=ml_dtypes.bfloat16)
    negmarg = np.full((128, 1), -MARGIN, dtype=np.float32)
    maskt = _make_mask()
    masktT = np.ascontiguousarray(maskt.T)

    WqT = Wq.T * scale                    # [H, 16*D], scale folded into q path
    WkT = np.ascontiguousarray(Wk.T)
    WvT_bf = Wv.T.astype(ml_dtypes.bfloat16)
    WoT_bf = Wo.T.astype(ml_dtypes.bfloat16)   # [H(in=ctx), H(out)] rows = ctx hidden

    in_maps = []
    for c in range(N_CORES):
        b, g = divmod(c, NH)
        js = slice(g * HG, (g + 1) * HG)
        xT_b = np.ascontiguousarray(x[b].T)
        in_maps.append({
            "xT": xT_b,
            "xbfT": xT_b.astype(ml_dtypes.bfloat16),
            "wqT": np.ascontiguousarray(WqT[:, js]).astype(np.float32),
            "wkT": np.ascontiguousarray(WkT[:, js]),
            "wvT": np.ascontiguousarray(WvT_bf[:, js]),
            "woT": np.ascontiguousarray(WoT_bf[js, :]),
            "cosT": cosT,
            "sinT": sinT,
            "rT": rT,
            "onesf": onesf,
            "onescol": onescol,
            "oneskk": oneskk,
            "negmarg": negmarg,
            "mask": maskt,
            "maskT": masktT,
        })

    LAST_RESULTS = run_bass_kernel_spmd(
        nc, in_maps, core_ids=list(range(N_CORES)), trace=TRACE
    )
    res = LAST_RESULTS.results

    out = np.zeros((B, S, H), dtype=np.float32)
    for c in range(N_CORES):
        b = c // NH
        out[b] += res[c]["out"]
    return out
